# revision 15
# baseline (speedup 1.0000x reference)
"""Trainium2 Bass kernel for nn_CNN_24472723653055 (AdderNet CNN).

Data-parallel over 8 NeuronCores: 2 images per core. BatchNorm batch stats and
the global LayerNorm stats are synchronized with small AllReduces.

adder2d(out[p,c] = -sum_k |x[p,k] - w[c,k]|) per layer:
  |w-x| = 2*relu(w-x) - w + x, so
  sum_k |w-x| = 2*sum_k relu(w-x) - wsum[c] + xsum[p].
  * weights live in SBUF transposed as [k_partition, c_free] tiles
  * per output position p and k-block: one fused DVE tensor_scalar
    (subtract, max 0) or one ScalarE Relu(w - x) produces D[k, c]
    (the per-partition scalar/bias is an activation column)
  * TensorE reduces 2*D over the k partitions with a one-hot 2.0 stationary
    [128, 32] column (col-tiled), accumulating psum[p_row, c] over k-blocks
  * the rank-1 corrections fold into the psum evacuation:
    Y = psum + xsum[p] (per-partition scalar) + (-wsum) (broadcast tile)
"""

import sys

sys.path.insert(0, "/opt/trn_rl_repo")

import numpy as np

N_CORES = 8
N_LOC = 2            # images per core
N_TOT = 16

C1, H1, W1 = 128, 196, 3
HO1 = 96
P1 = N_LOC * HO1 * W1          # 576
C2, HO2 = 256, 46
P2 = N_LOC * HO2 * W1          # 276
C3, HO3, WO3 = 384, 21, 2
P3 = N_LOC * HO3 * WO3         # 84
TAPS1, TAPS2 = 6, 6
TAPS3 = 12                     # (kh=6) x (kw=2)
KB3 = 24                       # 2 ci-blocks x 12 taps

EPS_BN = 1e-5
EPS_LN = 1e-5
EPS_L2 = 1e-12

# fraction of (p, tap) work routed to ScalarE instead of VectorE
ACT_FRAC_L2 = 0.33
ACT_FRAC_L3 = 0.33

_BUILD_CACHE = {}


def _route_act(ordinal: int, frac: float) -> bool:
    if frac <= 0.0:
        return False
    period = max(1, round(1.0 / frac))
    return ordinal % period == 0


def build_program():
    import concourse.bass as bass
    import concourse.bacc as bacc
    import concourse.tile as tile
    import concourse.mybir as mybir
    from concourse import masks

    dt = mybir.dt
    f32 = dt.float32
    Alu = mybir.AluOpType
    Act = mybir.ActivationFunctionType

    nc = bacc.Bacc("TRN2", target_bir_lowering=False, debug=False,
                   num_devices=N_CORES)

    # ------------------------------------------------------------------ I/O
    x_in = nc.dram_tensor("x_in", [1, N_LOC * H1 * W1], f32, kind="ExternalInput").ap()
    nw1t = nc.dram_tensor("nw1t", [1, TAPS1 * C1], f32, kind="ExternalInput").ap()
    w2t = nc.dram_tensor("w2t", [128, TAPS2 * C2], f32, kind="ExternalInput").ap()
    w3t = nc.dram_tensor("w3t", [128, KB3 * C3], f32, kind="ExternalInput").ap()
    wfcp = nc.dram_tensor("wfcp", [128, 6 * 3 * 42], f32, kind="ExternalInput").ap()
    g1_d = nc.dram_tensor("g1_d", [C1], f32, kind="ExternalInput").ap()
    b1_d = nc.dram_tensor("b1_d", [C1], f32, kind="ExternalInput").ap()
    g2_d = nc.dram_tensor("g2_d", [C2], f32, kind="ExternalInput").ap()
    b2_d = nc.dram_tensor("b2_d", [C2], f32, kind="ExternalInput").ap()
    g3_d = nc.dram_tensor("g3_d", [C3], f32, kind="ExternalInput").ap()
    b3_d = nc.dram_tensor("b3_d", [C3], f32, kind="ExternalInput").ap()
    bfc_d = nc.dram_tensor("bfc_d", [6], f32, kind="ExternalInput").ap()
    out_d = nc.dram_tensor("out", [1, N_LOC * 6], f32, kind="ExternalOutput").ap()

    groups = [list(range(N_CORES))]

    with tile.TileContext(nc) as tc:
        with tc.tile_pool(name="weights", bufs=1) as wp, \
             tc.tile_pool(name="acts", bufs=1) as ap_pool, \
             tc.tile_pool(name="consts", bufs=1) as cp, \
             tc.tile_pool(name="smalls", bufs=1) as sp, \
             tc.tile_pool(name="dram", bufs=1, space="DRAM") as dram:

            # ---------------------------------------------------- constants
            twos_col = cp.tile([128, 63], f32)      # one-hot 2.0 col bank (E32)
            nc.vector.memset(twos_col[:], 0.0)
            nc.vector.memset(twos_col[:, 31:32], 2.0)
            ones_row = cp.tile([1, 512], f32)       # K=1 all-ones rows
            nc.vector.memset(ones_row[:], 1.0)
            ones_k = cp.tile([128, 1], f32)         # stats reduction lhsT
            nc.vector.memset(ones_k[:], 1.0)
            ident = cp.tile([128, 128], f32)
            masks.make_identity(nc, ident[:])

            # ---------------------------------------------------- weight DMAs
            x_sb = wp.tile([1, N_LOC * H1 * W1], f32)
            nc.sync.dma_start(x_sb[:], x_in)
            nw1_sb = wp.tile([1, TAPS1 * C1], f32)
            nc.sync.dma_start(nw1_sb[:], nw1t)
            w2t_sb = wp.tile([128, TAPS2 * C2], f32)
            nc.sync.dma_start(w2t_sb[:], w2t)
            w3t_sb = wp.tile([128, KB3 * C3], f32)
            nc.sync.dma_start(w3t_sb[:], w3t)
            wfc_sb = wp.tile([128, 6 * 3 * 42], f32)
            nc.sync.dma_start(wfc_sb[:], wfcp)
            bfc_sb = sp.tile([1, 6], f32)
            nc.gpsimd.dma_start(bfc_sb[:], bfc_d.rearrange("(one j) -> one j", one=1))

            gb1 = sp.tile([128, 2], f32)
            nc.gpsimd.dma_start(gb1[:, 0:1], g1_d.rearrange("(p one) -> p one", one=1))
            nc.gpsimd.dma_start(gb1[:, 1:2], b1_d.rearrange("(p one) -> p one", one=1))
            gb2 = [sp.tile([128, 2], f32, name=f"gb2_{cb}") for cb in range(2)]
            gb3 = [sp.tile([128, 2], f32, name=f"gb3_{cb}") for cb in range(3)]
            for cb in range(2):
                nc.gpsimd.dma_start(gb2[cb][:, 0:1],
                                    g2_d[cb * 128:(cb + 1) * 128].rearrange("(p one) -> p one", one=1))
                nc.gpsimd.dma_start(gb2[cb][:, 1:2],
                                    b2_d[cb * 128:(cb + 1) * 128].rearrange("(p one) -> p one", one=1))
            for cb in range(3):
                nc.gpsimd.dma_start(gb3[cb][:, 0:1],
                                    g3_d[cb * 128:(cb + 1) * 128].rearrange("(p one) -> p one", one=1))
                nc.gpsimd.dma_start(gb3[cb][:, 1:2],
                                    b3_d[cb * 128:(cb + 1) * 128].rearrange("(p one) -> p one", one=1))

            # persistent activation tensors
            accr = ap_pool.tile([128, P1], f32)     # layer1 sum relu(x-w), [c1, p1]
            acc1 = ap_pool.tile([128, P1], f32)     # layer1 sum |x-w|
            act1 = ap_pool.tile([128, P1], f32)
            nact1 = ap_pool.tile([128, P1], f32)
            y2 = ap_pool.tile([128, 3 * C2], f32)   # layer2 raw, [p-rows, (pb, c)]
            act2 = [ap_pool.tile([128, P2], f32, name=f"act2_{cb}") for cb in range(2)]
            nact2 = [ap_pool.tile([128, P2], f32, name=f"nact2_{cb}") for cb in range(2)]
            y3 = ap_pool.tile([128, C3], f32)       # layer3 raw, [p3-rows, c3]
            act3 = ap_pool.tile([128, 3 * P3], f32)  # [ci, (cb, p3)]
            wb2 = ap_pool.tile([128, C2], f32)      # -wsum2 broadcast
            wb3 = ap_pool.tile([128, C3], f32)      # -wsum3 broadcast
            xs2col = sp.tile([128, 3], f32)         # xsum2 as columns per p-block
            xs3col = sp.tile([128, 1], f32)

            # ---------------------------------------------------- helpers
            def allreduce(sbuf_src_aps, widths, name):
                total = sum(a.shape[0] * w for a, w in zip(sbuf_src_aps, widths))
                cin = dram.tile([1, total], f32, name=f"cc_in_{name}")
                cout = dram.tile([1, total], f32, name=f"cc_out_{name}")
                off = 0
                for a, w in zip(sbuf_src_aps, widths):
                    n = a.shape[0] * w
                    nc.gpsimd.dma_start(
                        cin[0:1, off:off + n].rearrange("one (p w) -> (one p) w", w=w), a)
                    off += n
                nc.gpsimd.collective_compute(
                    "AllReduce", Alu.add, replica_groups=groups,
                    ins=[cin.opt()], outs=[cout.opt()])
                return cout

            def bn_affine(st_sum, st_sq, gb, n_bn, name):
                t_pool = sp
                mean = t_pool.tile([128, 1], f32, name=f"{name}_mean")
                msq = t_pool.tile([128, 1], f32, name=f"{name}_msq")
                m2 = t_pool.tile([128, 1], f32, name=f"{name}_m2")
                tv = t_pool.tile([128, 1], f32, name=f"{name}_tv")
                s_ = t_pool.tile([128, 1], f32, name=f"{name}_s")
                r0 = t_pool.tile([128, 1], f32, name=f"{name}_r0")
                r0sq = t_pool.tile([128, 1], f32, name=f"{name}_r0sq")
                av = t_pool.tile([128, 1], f32, name=f"{name}_av")
                bv = t_pool.tile([128, 1], f32, name=f"{name}_bv")
                rr = t_pool.tile([128, 1], f32, name=f"{name}_rr")
                gr = t_pool.tile([128, 1], f32, name=f"{name}_gr")
                scale = t_pool.tile([128, 1], f32, name=f"{name}_scale")
                bias = t_pool.tile([128, 1], f32, name=f"{name}_bias")
                inv = 1.0 / n_bn
                nc.vector.tensor_scalar(out=mean[:], in0=st_sum, scalar1=inv,
                                        scalar2=None, op0=Alu.mult)
                nc.vector.tensor_scalar(out=msq[:], in0=st_sq, scalar1=inv,
                                        scalar2=None, op0=Alu.mult)
                nc.vector.tensor_tensor(out=m2[:], in0=mean[:], in1=mean[:], op=Alu.mult)
                nc.vector.scalar_tensor_tensor(out=tv[:], in0=msq[:], scalar=EPS_BN,
                                               in1=m2[:], op0=Alu.add, op1=Alu.subtract)
                nc.scalar.activation(out=s_[:], in_=tv[:], func=Act.Sqrt)
                nc.vector.reciprocal(out=r0[:], in_=s_[:])
                # one Newton step for rsqrt accuracy: r = r0*(1.5 - 0.5*tv*r0^2)
                nc.vector.tensor_tensor(out=r0sq[:], in0=r0[:], in1=r0[:], op=Alu.mult)
                nc.vector.tensor_tensor(out=av[:], in0=tv[:], in1=r0sq[:], op=Alu.mult)
                nc.vector.tensor_scalar(out=bv[:], in0=av[:], scalar1=-0.5,
                                        scalar2=1.5, op0=Alu.mult, op1=Alu.add)
                nc.vector.tensor_tensor(out=rr[:], in0=r0[:], in1=bv[:], op=Alu.mult)
                nc.vector.tensor_tensor(out=gr[:], in0=gb[:, 0:1], in1=rr[:], op=Alu.mult)
                nc.vector.tensor_scalar(out=scale[:], in0=gr[:], scalar1=-1.0,
                                        scalar2=None, op0=Alu.mult)
                nc.vector.scalar_tensor_tensor(out=bias[:], in0=gr[:], scalar=mean[:],
                                               in1=gb[:, 1:2], op0=Alu.mult, op1=Alu.add)
                return scale, bias

            # =================================================== layer 1
            xv = x_sb.rearrange("one (n h w) -> one n h w", n=N_LOC, h=H1, w=W1)
            with tc.tile_pool(name="ps1", bufs=2, space="PSUM") as ps1, \
                 tc.tile_pool(name="pre1", bufs=2, space="PSUM") as pre1:
                for half in range(N_LOC):
                    for tap in range(TAPS1):
                        pk = ps1.tile([128, HO1 * W1], f32, tag="pk", name="pk")
                        xrow = xv[0:1, half, tap:tap + 2 * HO1 - 1:2, :]
                        nc.tensor.matmul(pk[:, :], lhsT=ones_row[0:1, 0:128],
                                         rhs=xrow, start=True, stop=False)
                        nc.tensor.matmul(pk[:, :],
                                         lhsT=nw1_sb[0:1, tap * C1:(tap + 1) * C1],
                                         rhs=ones_row[0:1, 0:HO1 * W1],
                                         start=False, stop=True)
                        dst = accr[:, half * HO1 * W1:(half + 1) * HO1 * W1]
                        if tap == 0:
                            nc.vector.tensor_scalar(out=dst, in0=pk[:, :], scalar1=0.0,
                                                    scalar2=None, op0=Alu.max)
                        else:
                            nc.vector.scalar_tensor_tensor(out=dst, in0=pk[:, :],
                                                           scalar=0.0, in1=dst,
                                                           op0=Alu.max, op1=Alu.add)
                # corrections: acc1 = 2*accr + ws1[c] - xs1[p]
                ws1 = sp.tile([1, C1], f32)
                nc.vector.tensor_scalar(out=ws1[:], in0=nw1_sb[0:1, 0:C1],
                                        scalar1=-1.0, scalar2=None, op0=Alu.mult)
                for tap in range(1, TAPS1):
                    nc.vector.scalar_tensor_tensor(
                        out=ws1[:], in0=nw1_sb[0:1, tap * C1:(tap + 1) * C1],
                        scalar=-1.0, in1=ws1[:], op0=Alu.mult, op1=Alu.add)
                pw1 = pre1.tile([128, 288], f32, tag="pre1", name="pw1")
                nc.tensor.matmul(pw1[:, 0:1], lhsT=ws1[0:1, :],
                                 rhs=ones_row[0:1, 0:1], start=True, stop=True)
                ws1col = sp.tile([128, 1], f32)
                nc.vector.tensor_copy(ws1col[:], pw1[:, 0:1])
                xs1 = sp.tile([1, P1], f32)
                xs1v = xs1.rearrange("one (n h w) -> one n h w", n=N_LOC, h=HO1, w=W1)
                nc.vector.tensor_scalar(out=xs1v[:], in0=xv[0:1, :, 0:2 * HO1 - 1:2, :],
                                        scalar1=0.0, scalar2=None, op0=Alu.add)
                for tap in range(1, TAPS1):
                    nc.vector.tensor_tensor(out=xs1v[:], in0=xs1v[:],
                                            in1=xv[0:1, :, tap:tap + 2 * HO1 - 1:2, :],
                                            op=Alu.add)
                nc.vector.tensor_scalar(out=acc1[:], in0=accr[:], scalar1=2.0,
                                        scalar2=ws1col[:], op0=Alu.mult, op1=Alu.add)
                for half in range(N_LOC):
                    pxb = pre1.tile([128, 288], f32, tag="pre1", name="pxb")
                    nc.tensor.matmul(pxb[:, :], lhsT=ones_row[0:1, 0:128],
                                     rhs=xs1[0:1, half * 288:(half + 1) * 288],
                                     start=True, stop=True)
                    sl = acc1[:, half * 288:(half + 1) * 288]
                    nc.vector.tensor_tensor(out=sl, in0=sl, in1=pxb[:, :],
                                            op=Alu.subtract)

            # BN1 stats (local): per-channel sum & sumsq over free dim
            s1_sum = sp.tile([128, 1], f32)
            s1_sq = sp.tile([128, 1], f32)
            scr1 = ap_pool.tile([128, P1], f32)
            nc.vector.tensor_scalar(out=scr1[:], in0=acc1[:], scalar1=0.0, scalar2=None,
                                    op0=Alu.add, op1=Alu.add, accum_out=s1_sum[:])
            nc.scalar.activation(out=scr1[:], in_=acc1[:], func=Act.Square,
                                 accum_out=s1_sq[:])
            cc1 = allreduce([s1_sum[:], s1_sq[:]], [1, 1], "bn1")
            st1 = sp.tile([128, 2], f32)
            nc.gpsimd.dma_start(st1[:, 0:1],
                                cc1[0:1, 0:128].rearrange("one (p w) -> (one p) w", w=1))
            nc.gpsimd.dma_start(st1[:, 1:2],
                                cc1[0:1, 128:256].rearrange("one (p w) -> (one p) w", w=1))
            sc1, bi1 = bn_affine(st1[:, 0:1], st1[:, 1:2], gb1, N_TOT * HO1 * W1, "bn1")
            nc.scalar.activation(out=act1[:], in_=acc1[:], func=Act.Relu,
                                 scale=sc1[:], bias=bi1[:])
            nc.vector.tensor_scalar(out=nact1[:], in0=act1[:], scalar1=-1.0,
                                    scalar2=None, op0=Alu.mult)

            # =================================================== layer 2
            with tc.tile_pool(name="ps2", bufs=2, space="PSUM") as ps2, \
                 tc.tile_pool(name="ps2s", bufs=1, space="PSUM") as ps2s, \
                 tc.tile_pool(name="pre2", bufs=2, space="PSUM") as pre2, \
                 tc.tile_pool(name="d2p", bufs=8) as d2p, \
                 tc.tile_pool(name="sq2p", bufs=2) as sq2p:
                # ---- corrections prelude
                pw2 = pre2.tile([128, C2], f32, tag="pre2", name="pw2")
                for tap in range(TAPS2):
                    nc.tensor.matmul(pw2[0:1, :], lhsT=ones_k[:, 0:1],
                                     rhs=w2t_sb[:, tap * C2:(tap + 1) * C2],
                                     start=(tap == 0), stop=(tap == TAPS2 - 1))
                negw2 = sp.tile([1, C2], f32)
                nc.vector.tensor_scalar(out=negw2[:], in0=pw2[0:1, :], scalar1=-1.0,
                                        scalar2=None, op0=Alu.mult)
                pb2 = pre2.tile([128, C2], f32, tag="pre2", name="pb2")
                nc.tensor.matmul(pb2[:, :], lhsT=ones_row[0:1, 0:128],
                                 rhs=negw2[0:1, :], start=True, stop=True)
                nc.vector.tensor_copy(wb2[:], pb2[:, :])
                cs1 = sp.tile([1, P1], f32)
                for half in range(N_LOC):
                    pcs = pre2.tile([128, 288], f32, tag="pre2", name="pcs")
                    nc.tensor.matmul(pcs[0:1, 0:288], lhsT=ones_k[:, 0:1],
                                     rhs=act1[:, half * 288:(half + 1) * 288],
                                     start=True, stop=True)
                    nc.vector.tensor_copy(cs1[0:1, half * 288:(half + 1) * 288],
                                          pcs[0:1, 0:288])
                xs2 = sp.tile([1, P2], f32)
                cs1v = cs1.rearrange("one (n h w) -> one n h w", n=N_LOC, h=HO1, w=W1)
                xs2v = xs2.rearrange("one (n h w) -> one n h w", n=N_LOC, h=HO2, w=W1)
                nc.vector.tensor_scalar(out=xs2v[:], in0=cs1v[0:1, :, 0:2 * HO2 - 1:2, :],
                                        scalar1=0.0, scalar2=None, op0=Alu.add)
                for tap in range(1, TAPS2):
                    nc.vector.tensor_tensor(out=xs2v[:], in0=xs2v[:],
                                            in1=cs1v[0:1, :, tap:tap + 2 * HO2 - 1:2, :],
                                            op=Alu.add)
                px2 = pre2.tile([128, C2], f32, tag="pre2", name="px2")
                for pb in range(3):
                    rows = 128 if pb < 2 else P2 - 256
                    nc.tensor.matmul(px2[0:rows, pb:pb + 1],
                                     lhsT=xs2[0:1, pb * 128:pb * 128 + rows],
                                     rhs=ones_row[0:1, 0:1], start=True, stop=True,
                                     skip_group_check=True)
                    nc.vector.tensor_copy(xs2col[0:rows, pb:pb + 1],
                                          px2[0:rows, pb:pb + 1])

                # ---- main loop
                st2_sum = ps2s.tile([1, C2], f32)
                st2_sq = ps2s.tile([1, C2], f32)
                ord2 = 0
                for pb in range(3):
                    rows = 128 if pb < 2 else P2 - 256
                    pt = ps2.tile([128, C2], f32, tag="pt2", name="pt2")
                    for r in range(rows):
                        p = pb * 128 + r
                        n, rem = divmod(p, HO2 * W1)
                        ho, wo = divmod(rem, W1)
                        j32, idx = divmod(r, 32)
                        grp_rows = min(rows - 32 * j32, 32)
                        for tap in range(TAPS2):
                            col = n * (HO1 * W1) + (2 * ho + tap) * W1 + wo
                            dtile = d2p.tile([128, C2], f32, tag="d2", name="d2")
                            if _route_act(ord2, ACT_FRAC_L2):
                                nc.scalar.activation(
                                    out=dtile[:], in_=w2t_sb[:, tap * C2:(tap + 1) * C2],
                                    func=Act.Relu, bias=nact1[:, col:col + 1])
                            else:
                                nc.vector.tensor_scalar(
                                    out=dtile[:], in0=w2t_sb[:, tap * C2:(tap + 1) * C2],
                                    scalar1=act1[:, col:col + 1], scalar2=0.0,
                                    op0=Alu.subtract, op1=Alu.max)
                            ord2 += 1
                            first = (r == 32 * j32) and tap == 0
                            last = (r == 32 * j32 + grp_rows - 1) and tap == TAPS2 - 1
                            nc.tensor.matmul(pt[32 * j32:32 * j32 + 32, :],
                                             lhsT=twos_col[:, 31 - idx:63 - idx],
                                             rhs=dtile[:], start=first, stop=last,
                                             tile_position=(0, 32 * j32))
                    # evacuate with corrections + stats
                    ysl = y2[0:rows, pb * C2:(pb + 1) * C2]
                    nc.vector.scalar_tensor_tensor(
                        out=ysl, in0=pt[0:rows, :], scalar=xs2col[0:rows, pb:pb + 1],
                        in1=wb2[0:rows, :], op0=Alu.add, op1=Alu.add)
                    sq_t = sq2p.tile([128, C2], f32, tag="sq2", name="sq2")
                    nc.scalar.activation(out=sq_t[0:rows, :], in_=ysl, func=Act.Square)
                    nc.tensor.matmul(st2_sum[0:1, :], lhsT=ones_k[0:rows, 0:1],
                                     rhs=ysl, start=(pb == 0), stop=(pb == 2),
                                     skip_group_check=True)
                    nc.tensor.matmul(st2_sq[0:1, :], lhsT=ones_k[0:rows, 0:1],
                                     rhs=sq_t[0:rows, :], start=(pb == 0), stop=(pb == 2),
                                     skip_group_check=True)
                st2_sb = sp.tile([1, 2 * C2], f32)
                nc.vector.tensor_copy(st2_sb[0:1, 0:C2], st2_sum[0:1, :])
                nc.vector.tensor_copy(st2_sb[0:1, C2:2 * C2], st2_sq[0:1, :])
                cc2 = allreduce([st2_sb[0:1, :]], [2 * C2], "bn2")
            st2 = sp.tile([128, 4], f32)
            for cb in range(2):
                nc.gpsimd.dma_start(
                    st2[:, cb:cb + 1],
                    cc2[0:1, cb * 128:(cb + 1) * 128].rearrange("one (p w) -> (one p) w", w=1))
                nc.gpsimd.dma_start(
                    st2[:, 2 + cb:3 + cb],
                    cc2[0:1, C2 + cb * 128:C2 + (cb + 1) * 128].rearrange("one (p w) -> (one p) w", w=1))
            with tc.tile_pool(name="pst2", bufs=2, space="PSUM") as pst2:
                for cb in range(2):
                    sc2, bi2 = bn_affine(st2[:, cb:cb + 1], st2[:, 2 + cb:3 + cb],
                                         gb2[cb], N_TOT * HO2 * W1, f"bn2_{cb}")
                    ptr = pst2.tile([128, P2], f32, tag="pst2", name="pst2")
                    for pb in range(3):
                        rows = 128 if pb < 2 else P2 - 256
                        nc.tensor.transpose(
                            ptr[:, pb * 128:pb * 128 + rows],
                            y2[0:rows, pb * C2 + cb * 128:pb * C2 + (cb + 1) * 128],
                            ident[0:rows, 0:rows])
                    nc.scalar.activation(out=act2[cb][:], in_=ptr[:, :], func=Act.Relu,
                                         scale=sc2[:], bias=bi2[:])
                    nc.vector.tensor_scalar(out=nact2[cb][:], in0=act2[cb][:],
                                            scalar1=-1.0, scalar2=None, op0=Alu.mult)

            # =================================================== layer 3
            with tc.tile_pool(name="ps3", bufs=1, space="PSUM") as ps3, \
                 tc.tile_pool(name="ps3s", bufs=1, space="PSUM") as ps3s, \
                 tc.tile_pool(name="pre3", bufs=2, space="PSUM") as pre3, \
                 tc.tile_pool(name="d3p", bufs=8) as d3p, \
                 tc.tile_pool(name="sq3p", bufs=1) as sq3p:
                # ---- corrections prelude
                pw3 = pre3.tile([128, C3], f32, tag="pre3", name="pw3")
                for kb in range(KB3):
                    nc.tensor.matmul(pw3[0:1, :], lhsT=ones_k[:, 0:1],
                                     rhs=w3t_sb[:, kb * C3:(kb + 1) * C3],
                                     start=(kb == 0), stop=(kb == KB3 - 1))
                negw3 = sp.tile([1, C3], f32)
                nc.vector.tensor_scalar(out=negw3[:], in0=pw3[0:1, :], scalar1=-1.0,
                                        scalar2=None, op0=Alu.mult)
                pb3 = pre3.tile([128, C3], f32, tag="pre3", name="pb3")
                nc.tensor.matmul(pb3[:, :], lhsT=ones_row[0:1, 0:128],
                                 rhs=negw3[0:1, :], start=True, stop=True)
                nc.vector.tensor_copy(wb3[:], pb3[:, :])
                cs2 = sp.tile([1, 2 * P2], f32)
                for cb in range(2):
                    pcs2 = pre3.tile([128, C3], f32, tag="pre3", name="pcs2")
                    nc.tensor.matmul(pcs2[0:1, 0:P2], lhsT=ones_k[:, 0:1],
                                     rhs=act2[cb][:], start=True, stop=True)
                    nc.vector.tensor_copy(cs2[0:1, cb * P2:(cb + 1) * P2],
                                          pcs2[0:1, 0:P2])
                xs3 = sp.tile([1, P3], f32)
                xs3v = xs3.rearrange("one (n h w) -> one n h w", n=N_LOC, h=HO3, w=WO3)
                first_x = True
                for cb in range(2):
                    csv = cs2[0:1, cb * P2:(cb + 1) * P2].rearrange(
                        "one (n h w) -> one n h w", n=N_LOC, h=HO2, w=W1)
                    for tap in range(TAPS3):
                        ki, kj = divmod(tap, 2)
                        view = csv[0:1, :, ki:ki + 2 * HO3 - 1:2, kj:kj + WO3]
                        if first_x:
                            nc.vector.tensor_scalar(out=xs3v[:], in0=view, scalar1=0.0,
                                                    scalar2=None, op0=Alu.add)
                            first_x = False
                        else:
                            nc.vector.tensor_tensor(out=xs3v[:], in0=xs3v[:],
                                                    in1=view, op=Alu.add)
                px3 = pre3.tile([128, C3], f32, tag="pre3", name="px3")
                nc.tensor.matmul(px3[0:P3, 0:1], lhsT=xs3[0:1, 0:P3],
                                 rhs=ones_row[0:1, 0:1], start=True, stop=True)
                nc.vector.tensor_copy(xs3col[0:P3, 0:1], px3[0:P3, 0:1])

                # ---- main loop
                pt3 = ps3.tile([128, C3], f32)
                ord3 = 0
                for p in range(P3):
                    n, rem = divmod(p, HO3 * WO3)
                    ho, wo = divmod(rem, WO3)
                    j32, idx = divmod(p, 32)
                    grp_rows = min(P3 - 32 * j32, 32)
                    for cib in range(2):
                        for tap in range(TAPS3):
                            ki, kj = divmod(tap, 2)
                            col = n * (HO2 * W1) + (2 * ho + ki) * W1 + (wo + kj)
                            kb = cib * TAPS3 + tap
                            dtile = d3p.tile([128, C3], f32, tag="d3", name="d3")
                            if _route_act(ord3, ACT_FRAC_L3):
                                nc.scalar.activation(
                                    out=dtile[:], in_=w3t_sb[:, kb * C3:(kb + 1) * C3],
                                    func=Act.Relu, bias=nact2[cib][:, col:col + 1])
                            else:
                                nc.vector.tensor_scalar(
                                    out=dtile[:], in0=w3t_sb[:, kb * C3:(kb + 1) * C3],
                                    scalar1=act2[cib][:, col:col + 1], scalar2=0.0,
                                    op0=Alu.subtract, op1=Alu.max)
                            ord3 += 1
                            first = (p == 32 * j32) and kb == 0
                            last = (p == 32 * j32 + grp_rows - 1) and kb == KB3 - 1
                            nc.tensor.matmul(pt3[32 * j32:32 * j32 + 32, :],
                                             lhsT=twos_col[:, 31 - idx:63 - idx],
                                             rhs=dtile[:], start=first, stop=last,
                                             tile_position=(0, 32 * j32))
                nc.vector.scalar_tensor_tensor(
                    out=y3[0:P3, :], in0=pt3[0:P3, :], scalar=xs3col[0:P3, 0:1],
                    in1=wb3[0:P3, :], op0=Alu.add, op1=Alu.add)
                st3_sum = ps3s.tile([1, C3], f32)
                st3_sq = ps3s.tile([1, C3], f32)
                sq3 = sq3p.tile([128, C3], f32)
                nc.scalar.activation(out=sq3[0:P3, :], in_=y3[0:P3, :], func=Act.Square)
                nc.tensor.matmul(st3_sum[0:1, :], lhsT=ones_k[0:P3, 0:1],
                                 rhs=y3[0:P3, :], start=True, stop=True)
                nc.tensor.matmul(st3_sq[0:1, :], lhsT=ones_k[0:P3, 0:1],
                                 rhs=sq3[0:P3, :], start=True, stop=True)
                st3_sb = sp.tile([1, 2 * C3], f32)
                nc.vector.tensor_copy(st3_sb[0:1, 0:C3], st3_sum[0:1, :])
                nc.vector.tensor_copy(st3_sb[0:1, C3:2 * C3], st3_sq[0:1, :])
                cc3 = allreduce([st3_sb[0:1, :]], [2 * C3], "bn3")
            st3 = sp.tile([128, 6], f32)
            for cb in range(3):
                nc.gpsimd.dma_start(
                    st3[:, cb:cb + 1],
                    cc3[0:1, cb * 128:(cb + 1) * 128].rearrange("one (p w) -> (one p) w", w=1))
                nc.gpsimd.dma_start(
                    st3[:, 3 + cb:4 + cb],
                    cc3[0:1, C3 + cb * 128:C3 + (cb + 1) * 128].rearrange("one (p w) -> (one p) w", w=1))
            with tc.tile_pool(name="pst3", bufs=2, space="PSUM") as pst3:
                for cb in range(3):
                    sc3, bi3 = bn_affine(st3[:, cb:cb + 1], st3[:, 3 + cb:4 + cb],
                                         gb3[cb], N_TOT * HO3 * WO3, f"bn3_{cb}")
                    ptr3 = pst3.tile([128, P3], f32, tag="pst3", name="pst3")
                    nc.tensor.transpose(ptr3[:, 0:P3],
                                        y3[0:P3, cb * 128:(cb + 1) * 128],
                                        ident[0:P3, 0:P3])
                    nc.scalar.activation(out=act3[:, cb * P3:(cb + 1) * P3],
                                         in_=ptr3[:, 0:P3], func=Act.Relu,
                                         scale=sc3[:], bias=bi3[:])

            # =================================================== FC + LN + L2
            with tc.tile_pool(name="psfc", bufs=1, space="PSUM") as psfc_p, \
                 tc.tile_pool(name="fcp", bufs=2) as fcp:
                fcacc = sp.tile([128, 12], f32)
                for jj in range(6):
                    for n in range(N_LOC):
                        prod = fcp.tile([128, 3 * 42], f32, tag="prod", name="prod")
                        a3v = act3.rearrange("p (cb q) -> p cb q", cb=3)[:, :, n * 42:(n + 1) * 42]
                        wv = wfc_sb.rearrange("p (j cb q) -> p j cb q", j=6, cb=3)[:, jj]
                        nc.vector.scalar_tensor_tensor(
                            out=prod[:], in0=a3v, scalar=0.0, in1=wv,
                            op0=Alu.add, op1=Alu.mult,
                            accum_out=fcacc[:, jj * 2 + n:jj * 2 + n + 1])
                psfc = psfc_p.tile([1, 12], f32)
                nc.tensor.matmul(psfc[0:1, :], lhsT=ones_k[:, 0:1], rhs=fcacc[:],
                                 start=True, stop=True)
                h12 = sp.tile([1, 12], f32)
                h12v = h12.rearrange("one (j n) -> one j n", n=N_LOC)
                psv = psfc.rearrange("one (j n) -> one j n", n=N_LOC)
                for n in range(N_LOC):
                    nc.vector.tensor_tensor(out=h12v[:, :, n], in0=psv[:, :, n],
                                            in1=bfc_sb[:], op=Alu.add)
                # LN stats
                lnS = sp.tile([1, 1], f32)
                lnQ = sp.tile([1, 1], f32)
                scrl = sp.tile([1, 12], f32)
                nc.vector.tensor_scalar(out=scrl[:], in0=h12[:], scalar1=0.0,
                                        scalar2=None, op0=Alu.add, op1=Alu.add,
                                        accum_out=lnS[:])
                nc.scalar.activation(out=scrl[:], in_=h12[:], func=Act.Square,
                                     accum_out=lnQ[:])
                ccl = allreduce([lnS[:], lnQ[:]], [1, 1], "ln")
                stl = sp.tile([1, 2], f32)
                nc.gpsimd.dma_start(stl[:], ccl[0:1, 0:2])
                mu = sp.tile([1, 1], f32)
                qv = sp.tile([1, 1], f32)
                mu2 = sp.tile([1, 1], f32)
                tvl = sp.tile([1, 1], f32)
                sl_ = sp.tile([1, 1], f32)
                rl0 = sp.tile([1, 1], f32)
                rl0sq = sp.tile([1, 1], f32)
                avl = sp.tile([1, 1], f32)
                bvl = sp.tile([1, 1], f32)
                rl = sp.tile([1, 1], f32)
                inv_tot = 1.0 / (N_TOT * 6)
                nc.vector.tensor_scalar(out=mu[:], in0=stl[:, 0:1], scalar1=inv_tot,
                                        scalar2=None, op0=Alu.mult)
                nc.vector.tensor_scalar(out=qv[:], in0=stl[:, 1:2], scalar1=inv_tot,
                                        scalar2=None, op0=Alu.mult)
                nc.vector.tensor_tensor(out=mu2[:], in0=mu[:], in1=mu[:], op=Alu.mult)
                nc.vector.scalar_tensor_tensor(out=tvl[:], in0=qv[:], scalar=EPS_LN,
                                               in1=mu2[:], op0=Alu.add, op1=Alu.subtract)
                nc.scalar.activation(out=sl_[:], in_=tvl[:], func=Act.Sqrt)
                nc.vector.reciprocal(out=rl0[:], in_=sl_[:])
                nc.vector.tensor_tensor(out=rl0sq[:], in0=rl0[:], in1=rl0[:], op=Alu.mult)
                nc.vector.tensor_tensor(out=avl[:], in0=tvl[:], in1=rl0sq[:], op=Alu.mult)
                nc.vector.tensor_scalar(out=bvl[:], in0=avl[:], scalar1=-0.5,
                                        scalar2=1.5, op0=Alu.mult, op1=Alu.add)
                nc.vector.tensor_tensor(out=rl[:], in0=rl0[:], in1=bvl[:], op=Alu.mult)
                y12 = sp.tile([1, 12], f32)
                nc.vector.tensor_scalar(out=y12[:], in0=h12[:], scalar1=mu[:],
                                        scalar2=rl[:], op0=Alu.subtract, op1=Alu.mult)
                ysq = sp.tile([1, 12], f32)
                nc.scalar.activation(out=ysq[:], in_=y12[:], func=Act.Square)
                out12 = sp.tile([1, 12], f32)
                y12v = y12.rearrange("one (j n) -> one j n", n=N_LOC)
                ysqv = ysq.rearrange("one (j n) -> one j n", n=N_LOC)
                o12v = out12.rearrange("one (j n) -> one j n", n=N_LOC)
                for n in range(N_LOC):
                    nrm = sp.tile([1, 1], f32, name=f"nrm_{n}")
                    srt = sp.tile([1, 1], f32, name=f"srt_{n}")
                    mx = sp.tile([1, 1], f32, name=f"mx_{n}")
                    ivn = sp.tile([1, 1], f32, name=f"ivn_{n}")
                    scrn = sp.tile([1, 6], f32, name=f"scrn_{n}")
                    nc.vector.tensor_scalar(out=scrn[:], in0=ysqv[:, :, n], scalar1=0.0,
                                            scalar2=None, op0=Alu.add, op1=Alu.add,
                                            accum_out=nrm[:])
                    nc.scalar.activation(out=srt[:], in_=nrm[:], func=Act.Sqrt)
                    nc.vector.tensor_scalar(out=mx[:], in0=srt[:], scalar1=EPS_L2,
                                            scalar2=None, op0=Alu.max)
                    nc.vector.reciprocal(out=ivn[:], in_=mx[:])
                    nc.vector.tensor_scalar(out=o12v[:, :, n], in0=y12v[:, :, n],
                                            scalar1=ivn[:], scalar2=None, op0=Alu.mult)
                outnj = sp.tile([1, 12], f32)
                nc.vector.tensor_copy(
                    outnj.rearrange("one (n j) -> one n j", n=N_LOC),
                    out12.rearrange("one (j n) -> one n j", n=N_LOC))
                nc.gpsimd.dma_start(out_d, outnj[:])

    nc.compile()
    return nc


def _prep_inputs(inputs):
    """Host-side reshapes of the full inputs into per-core in_maps."""
    x = np.asarray(inputs["x"], np.float32)
    w1 = np.asarray(inputs["w1"], np.float32)
    w2 = np.asarray(inputs["w2"], np.float32)
    w3 = np.asarray(inputs["w3"], np.float32)
    Wfc = np.asarray(inputs["Wfc"], np.float32)

    nw1t = (-w1[:, 0, :, 0].T).reshape(1, TAPS1 * C1).copy()          # [1, 6*128]
    w2t = np.ascontiguousarray(w2[:, :, :, 0].transpose(1, 2, 0)).reshape(128, TAPS2 * C2)
    # w3: (384, 256, 6, 2) -> [ci_in_block, (cib, tap(i,j), c)]
    w3r = w3.transpose(1, 2, 3, 0).reshape(2, 128, TAPS3, C3)          # (cib, ci, tap, c)
    w3t = np.ascontiguousarray(w3r.transpose(1, 0, 2, 3)).reshape(128, KB3 * C3)
    # Wfc: (6, 16128) with k = c3*42 + ho*2 + wo -> [ci, (j, cb, howo)]
    wf = Wfc.reshape(6, 3, 128, 42)                                    # (j, cb, ci, howo)
    wfcp = np.ascontiguousarray(wf.transpose(2, 0, 1, 3)).reshape(128, 6 * 3 * 42)

    shared = {
        "nw1t": nw1t, "w2t": w2t, "w3t": w3t, "wfcp": wfcp,
        "g1_d": np.asarray(inputs["g1"], np.float32),
        "b1_d": np.asarray(inputs["b1"], np.float32),
        "g2_d": np.asarray(inputs["g2"], np.float32),
        "b2_d": np.asarray(inputs["b2"], np.float32),
        "g3_d": np.asarray(inputs["g3"], np.float32),
        "b3_d": np.asarray(inputs["b3"], np.float32),
        "bfc_d": np.asarray(inputs["bfc"], np.float32),
    }
    in_maps = []
    for i in range(N_CORES):
        m = dict(shared)
        m["x_in"] = np.ascontiguousarray(
            x[i * N_LOC:(i + 1) * N_LOC]).reshape(1, N_LOC * H1 * W1)
        in_maps.append(m)
    return in_maps


def _run(inputs, trace=False):
    if "nc" not in _BUILD_CACHE:
        _BUILD_CACHE["nc"] = build_program()
    nc = _BUILD_CACHE["nc"]
    from concourse import bass_utils
    in_maps = _prep_inputs(inputs)
    res = bass_utils.run_bass_kernel_spmd(
        nc, in_maps, core_ids=list(range(N_CORES)), trace=trace)
    out = np.concatenate(
        [np.asarray(r["out"]).reshape(N_LOC, 6) for r in res.results], axis=0)
    return out, res


def kernel(**inputs):
    return _run(inputs, trace=False)[0]
